# revision 31
# baseline (speedup 1.0000x reference)
"""Trainium2 Bass kernel for BF16IndexerBaseline (sparse_attention).

Computes, for q:(1,M,H,D) bf16, k:(1,N,D) bf16, weights:(H,M) bf16:

    index_score[b,m,n] = sum_h relu(q[b,m,h,:] . k[b,n,:]) * (weights[h,m]*D**-0.5)

Strategy (8 NeuronCores, SPMD, host-side sharding of m):
  - each core gets m-shard of 256 rows (2 m-tiles of 128), k replicated.
  - since weights >= 0, the per-(m,h) scale commutes with relu and is folded
    into the PSUM-eviction ops as a per-partition scalar (logits come out of
    the PE with m on partitions).
  - per (m-tile, n-chunk of 1024) unit: 16 heads x 2 matmuls (K=D=128
    contraction, stationary qT tile, moving kT) -> fp32 logits in PSUM
    ([128,1024] tiles, separate 2-buf pools for the A- and V-head roles).
  - epilogue split across engines (PSUM reads are the hard bottleneck:
    1 elem/lane/cyc per engine, ACT+DVE only):
      * 6 "chain" heads on VectorE via a runtime-registered fused custom
        DVE op RELU_SCALE_ADD: acc = relu(psum*s) + acc (fp32, 1 op/elem)
      * 10 heads on ScalarE: r = relu(psum*s) -> bf16 tiles; pair-sums
        2 on GpSimd + 3 on the DMA rings (SWDGE CCE accumulate, in-place
        SBUF->SBUF); upper tree + final combine on VectorE (bf16 2x).
  - k/q transposed once through the DMA xbar in pieces on both HWDGE
    queues so the first matmuls start early.
  - final: out = chain_acc + tree_root (fp32) -> DMA to DRAM.

Measured on 8x trn2 (NTFF profile, core 0): ~129 us vs ~55 us 8-core PE
roofline; epilogue(ACT/DVE/GpSimd)-bound, all within ~15% of each other.
"""

import os

os.environ.setdefault("MYCRO_LOCAL_CACHE", "1")

import numpy as np
import ml_dtypes
from contextlib import ExitStack

import concourse.bass as bass
import concourse.tile as tile
from concourse import bacc, mybir
from concourse.bass_utils import run_bass_kernel_spmd

# ---------------------------------------------------------------- problem dims
B = 1
M = 2048
H = 16
N = 4096
D = 128
N_CORES = 8
MS = M // N_CORES          # 256 rows of m per core
MT = MS // 128             # 2 m-tiles per core
FD = 1024                  # n-chunk (free dim) per epilogue op = 2 PSUM banks
NCH = N // FD              # 4 n-chunks
DVE_HEADS = int(os.environ.get("IDX_DVE_HEADS", "6"))   # fused-chain heads on VectorE
GPS_ADDS = int(os.environ.get("IDX_GPS_ADDS", "2"))     # tree adds on GpSimd per chunk
DMA_ADDS = int(os.environ.get("IDX_DMA_ADDS", "3"))     # tree pair-adds via DMA-accum per chunk
GPS_U = int(os.environ.get("IDX_GPS_U", "0"))           # upper-tree adds on GpSimd per chunk
FINAL_GPS = bool(int(os.environ.get("IDX_FINAL_GPS", "0")))  # final combine on GpSimd
DMA_FINAL = bool(int(os.environ.get("IDX_DMA_FINAL", "0")))  # root += acc via DMA-accum (slower on HW)
SEEDED = bool(int(os.environ.get("IDX_SEEDED", "0")))   # 1: tree root seeds chain (A then V)

BF16 = mybir.dt.bfloat16
F32 = mybir.dt.float32
# match the reference's bf16 rounding of SOFTMAX_SCALE
SCALE_BF16 = float(np.float32(np.array(D ** -0.5, dtype=ml_dtypes.bfloat16)))

# --------------------------------------------------- custom fused DVE op
# out = relu(in0 * s0) + in1   (s0 per-partition scalar [P,1])
import concourse.dve_ops as dve_ops
from concourse.dve_spec import Spec as _Spec, Src0 as _Src0, Src1 as _Src1, C0 as _C0
from concourse.dve_spec import relu as _relu, lower as _lower
from concourse.dve_uop import DveOpSpec as _DveOpSpec

_OP_NAME = "RELU_SCALE_ADD_ANT"


def _ref_relu_scale_add(in0, in1, s0, s1, imm2):
    x = np.nan_to_num(in0.astype(np.float32) * s0, nan=0.0, posinf=np.inf, neginf=-np.inf)
    return np.maximum(x, 0.0).astype(np.float32) + in1


def _register_relu_scale_add():
    for op in dve_ops.OPS:
        if op.name == _OP_NAME:
            return op
    spec = _Spec(body=_relu(_Src0 * _C0) + _Src1, reference=_ref_relu_scale_add)
    row = max(dve_ops._SUB_OPCODE_FOR_NAME.values()) + 1
    assert row < 0x20
    dve_ops._SUB_OPCODE_FOR_NAME[_OP_NAME] = row
    shas = {
        v: _DveOpSpec(name=_OP_NAME, opcode=row, uops=_lower(spec, ver=v), rd1_en=True).sha(v)
        for v in ("v3", "v4")
    }
    op = dve_ops.DveOp(_OP_NAME, spec, subdim=False, uops_sha=shas)
    dve_ops.OPS.append(op)
    dve_ops.CUSTOM_DVE_SPECS[_OP_NAME] = spec
    return op


RELU_SCALE_ADD = _register_relu_scale_add()


def _head_roles(v_heads: int) -> list[str]:
    if SEEDED:
        # A-heads first, then the V chain (tree root seeds its first op)
        return ["A"] * (H - v_heads) + ["V"] * v_heads
    # V (fused DVE chain) spread evenly among A (ACT evict)
    roles = ["A"] * H
    if v_heads > 0:
        step = H / v_heads
        for i in range(v_heads):
            roles[min(H - 1, int((i + 0.7) * step))] = "V"
    assert roles.count("V") == v_heads
    return roles


# ------------------------------------------------------------------ kernel IR
def _emit(ctx: ExitStack, tc: "tile.TileContext", q_d, k_d, w_d, o_d):
    nc = tc.nc
    AOp = mybir.AluOpType
    roles = _head_roles(DVE_HEADS)

    const = ctx.enter_context(tc.tile_pool(name="const", bufs=1))
    psA = ctx.enter_context(tc.tile_pool(name="psA", bufs=2, space="PSUM"))
    psV = ctx.enter_context(tc.tile_pool(name="psV", bufs=2, space="PSUM"))
    rpool = ctx.enter_context(tc.tile_pool(name="rpool", bufs=14))
    tpool = ctx.enter_context(tc.tile_pool(name="tpool", bufs=14))
    apool = ctx.enter_context(tc.tile_pool(name="apool", bufs=4))
    opool = ctx.enter_context(tc.tile_pool(name="opool", bufs=3))

    # scale vectors first (tiny; ready before the first ACT eviction)
    w_raw = const.tile([128, H * MT], BF16)       # [p, h*MT+mt]
    nc.sync.dma_start(out=w_raw[:], in_=w_d)
    s_bf = const.tile([128, H * MT], BF16)        # bf16(w * bf16(scale)) == ref q_s
    nc.vector.tensor_scalar_mul(s_bf[:], w_raw[:], SCALE_BF16)
    s_f = const.tile([128, H * MT], F32)
    nc.vector.tensor_copy(s_f[:], s_bf[:])

    # transposes via DMA xbar, split into pieces alternating across the two
    # HWDGE queues so the first pieces of BOTH tensors land asap and the
    # first matmuls can start early
    kT = const.tile([128, N], BF16)
    qT = const.tile([128, H * MS], BF16)          # columns: h*MS + m_local
    # mostly on the sync queue (issuing from the scalar queue costs ~1us of
    # the critical ACT engine track per piece); only k piece 0 goes on the
    # scalar queue so both tensors' first pieces transfer in parallel
    PIECES = 4
    kp, qp = N // PIECES, (H * MS) // PIECES
    nc.scalar.dma_start_transpose(out=kT[:, 0:kp], in_=k_d[0:kp, :])
    nc.sync.dma_start_transpose(out=qT[:, 0:qp], in_=q_d[0:qp, :])
    for i in range(1, PIECES):
        nc.sync.dma_start_transpose(
            out=kT[:, i * kp: (i + 1) * kp], in_=k_d[i * kp: (i + 1) * kp, :]
        )
        nc.sync.dma_start_transpose(
            out=qT[:, i * qp: (i + 1) * qp], in_=q_d[i * qp: (i + 1) * qp, :]
        )

    n_v = roles.count("V")
    for mt in range(MT):
        for nci in range(NCH):
            n0 = nci * FD
            uid = f"{mt}_{nci}"
            acc = apool.tile([128, FD], F32, tag="acc", name=f"acc_{uid}")
            stage = None
            if SEEDED or not DMA_FINAL:
                stage = opool.tile([128, FD], F32, tag="stage", name=f"stage_{uid}")
            r_tiles = []      # bf16 ACT-evicted tiles awaiting tree
            lvl0 = []         # GpSimd / DMA pair sums
            gps_left = GPS_ADDS
            dma_left = DMA_ADDS
            chain_i = 0
            root = None       # bf16 tree root (sum of all A heads)

            def _mk_head(h):
                pool = psV if roles[h] == "V" else psA
                pt = pool.tile([128, FD], F32, tag="logits", name=f"ps_{uid}_{h}")
                lhs = qT[:, h * MS + mt * 128: h * MS + mt * 128 + 128]
                for j in range(FD // 512):
                    nc.tensor.matmul(
                        pt[:, j * 512: (j + 1) * 512],
                        lhs,
                        kT[:, n0 + j * 512: n0 + (j + 1) * 512],
                        start=True,
                        stop=True,
                    )
                return pt, s_f[:, h * MT + mt: h * MT + mt + 1]

            def _emit_a(h):
                nonlocal r_tiles, gps_left, dma_left
                pt, s_ap = _mk_head(h)
                r = rpool.tile([128, FD], BF16, tag="r", name=f"r_{uid}_{h}")
                nc.scalar.activation(
                    r[:], pt[:], mybir.ActivationFunctionType.Relu, scale=s_ap
                )
                r_tiles.append(r)
                if len(r_tiles) >= 2 and dma_left > 0:
                    # in-place pair accumulate on the DMA rings (CCE)
                    nc.gpsimd.dma_start(
                        out=r_tiles[-2][:], in_=r_tiles[-1][:], accum_op=AOp.add
                    )
                    lvl0.append(r_tiles[-2])
                    r_tiles = r_tiles[:-2]
                    dma_left -= 1
                elif len(r_tiles) >= 2 and gps_left > 0:
                    t = tpool.tile([128, FD], BF16, tag="t", name=f"t{len(lvl0)}_{uid}")
                    nc.gpsimd.tensor_add(t[:], r_tiles[-2][:], r_tiles[-1][:])
                    lvl0.append(t)
                    r_tiles = r_tiles[:-2]
                    gps_left -= 1

            def _finish_tree():
                # finish the tree on VectorE (bf16 2x mode), GPS_U adds on GpSimd.
                # With DMA_FINAL the root is materialized in fp32 so it can be
                # CCE-accumulated into the fp32 DRAM output.
                work = lvl0 + r_tiles
                wi = 0
                gps_u = GPS_U
                while len(work) > 1:
                    nxt = []
                    last_lvl = len(work) <= 2
                    for i in range(0, len(work) - 1, 2):
                        dt = F32 if (DMA_FINAL and last_lvl) else BF16
                        tag = "tf" if dt is F32 else "t"
                        t = tpool.tile([128, FD], dt, tag=tag, name=f"u{wi}_{uid}")
                        wi += 1
                        eng = nc.gpsimd if gps_u > 0 else nc.vector
                        gps_u -= 1
                        eng.tensor_add(t[:], work[i][:], work[i + 1][:])
                        nxt.append(t)
                    if len(work) % 2:
                        nxt.append(work[-1])
                    work = nxt
                return work[0] if work else None

            def _emit_v(h, in1, out_ap):
                # out = relu(psum * s) + in1   (in1 None -> plain scaled relu)
                pt, s_ap = _mk_head(h)
                if in1 is None:
                    nc.vector.tensor_scalar(
                        out_ap[:], pt[:], s_ap, 0.0, op0=AOp.mult, op1=AOp.max
                    )
                else:
                    nc.vector._custom_dve(
                        RELU_SCALE_ADD, out=out_ap[:], in0=pt[:], in1=in1[:], s0=s_ap
                    )

            if SEEDED:
                for h, role in enumerate(roles):
                    if role == "A":
                        _emit_a(h)
                root = _finish_tree()
                prev = root
                for h, role in enumerate(roles):
                    if role != "V":
                        continue
                    last = chain_i == n_v - 1
                    out_ap = stage if (last and prev is not None or n_v == 1) else acc
                    _emit_v(h, prev, out_ap)
                    prev = out_ap
                    chain_i += 1
                if n_v == 0:
                    nc.vector.tensor_copy(stage[:], root[:])
            else:
                prev = None
                for h, role in enumerate(roles):
                    if role == "A":
                        _emit_a(h)
                    else:
                        _emit_v(h, prev, acc)
                        prev = acc
                        chain_i += 1
                root = _finish_tree()
                o_ap = o_d[mt * 128: (mt + 1) * 128, n0: n0 + FD]
                if DMA_FINAL and n_v and root is not None and root.dtype == F32:
                    # CCE-accumulate the tree root into the chain accumulator
                    # (SBUF->SBUF; accumulating into DRAM is broken on HW)
                    nc.gpsimd.dma_start(out=acc[:], in_=root[:], accum_op=AOp.add)
                    nc.sync.dma_start(out=o_ap, in_=acc[:])
                    continue
                if stage is None:
                    stage = opool.tile([128, FD], F32, tag="stage", name=f"stage_{uid}")
                if n_v and root is not None:
                    eng = nc.gpsimd if FINAL_GPS else nc.vector
                    eng.tensor_add(stage[:], acc[:], root[:])
                elif n_v:
                    nc.vector.tensor_copy(stage[:], acc[:])
                else:
                    nc.vector.tensor_copy(stage[:], root[:])
            nc.sync.dma_start(
                out=o_d[mt * 128: (mt + 1) * 128, n0: n0 + FD], in_=stage[:]
            )


_NC_CACHE = None


def _build():
    global _NC_CACHE
    if _NC_CACHE is not None:
        return _NC_CACHE
    nc = bacc.Bacc(
        "TRN2",
        target_bir_lowering=False,
        debug=False,
        enable_asserts=False,
        num_devices=N_CORES,
    )
    q_d = nc.dram_tensor("q", [H * MS, D], BF16, kind="ExternalInput").ap()
    k_d = nc.dram_tensor("k", [N, D], BF16, kind="ExternalInput").ap()
    w_d = nc.dram_tensor("w", [128, H * MT], BF16, kind="ExternalInput").ap()
    o_d = nc.dram_tensor("o", [MS, N], F32, kind="ExternalOutput").ap()
    with tile.TileContext(nc) as tc:
        with ExitStack() as ctx:
            _emit(ctx, tc, q_d, k_d, w_d, o_d)
    nc.compile()
    _NC_CACHE = (nc, q_d, k_d, w_d, o_d)
    return _NC_CACHE


def _shard_inputs(q, k, weights):
    q = np.asarray(q).astype(ml_dtypes.bfloat16, copy=False).reshape(M, H, D)
    k = np.ascontiguousarray(np.asarray(k).astype(ml_dtypes.bfloat16, copy=False).reshape(N, D))
    w = np.asarray(weights).astype(ml_dtypes.bfloat16, copy=False).reshape(H, M)
    in_maps = []
    for c in range(N_CORES):
        m0 = c * MS
        # rows ordered h-major: row = h*MS + m_local
        q_c = np.ascontiguousarray(q[m0: m0 + MS].transpose(1, 0, 2).reshape(H * MS, D))
        # w_c[p, h*MT+mt] = w[h, m0 + mt*128 + p]
        w_c = np.ascontiguousarray(
            w[:, m0: m0 + MS].reshape(H, MT, 128).transpose(2, 0, 1).reshape(128, H * MT)
        )
        in_maps.append({"q": q_c, "k": k, "w": w_c})
    return in_maps


LAST_RESULTS = None


def kernel(q, k, weights):
    global LAST_RESULTS
    nc, *_ = _build()
    in_maps = _shard_inputs(q, k, weights)
    trace = bool(int(os.environ.get("IDX_TRACE", "0")))
    res = run_bass_kernel_spmd(
        nc, in_maps, core_ids=list(range(N_CORES)), trace=trace
    )
    LAST_RESULTS = res
    out = np.empty((B, M, N), np.float32)
    for c in range(N_CORES):
        out[0, c * MS: (c + 1) * MS] = res.results[c]["o"]
    return out
